# revision 36
# baseline (speedup 1.0000x reference)
"""Trainium2 Bass kernel for nn_Head (sparse attention head).

Computation (per batch b):
    K = X @ Wk; Q = X @ Wq; V = X @ Wv                       # [T, HS]
    S = Q K^T / sqrt(HS)                                     # [T, T]
    A = softmax_row(where(dag==0, -inf, S))                  # row-wise over keys
    out[j, h] = sum_i A[i, j] V[i, h]   (transposed AV)      # [T, HS]
    return swish(out)

Sharding over 8 NeuronCores: core = (b, h) with b = batch (4), h = query-row
half (2).  The host computes the cheap O(T*D*HS) projections (4% of FLOPs)
and ships K^T/Q^T/V per core; the device does the O(T^2) work.  The key axis
is rotated per core by h*TH so DMA layouts are core-independent; the host
un-rotates the output.

Device phases per core:
  B: per 128-query block: QK matmuls (fp16, contraction 64) -> exp on ACT
     (scale 1/8) -> u = er*mask with the row-sum split three ways: DVE
     fused scalar_tensor_tensor for j < DVE_J, DVE tensor_tensor (2x mode)
     for the rest, whose sum comes from an ACT copy-accum pass (emitted one
     block late so it never stalls the exp stream) -> fold 1/l and VSCALE
     into vt.
  C: transposed-AV matmuls, PE-only, as four 32-wide output streams on
     distinct PE column quadrants (tile positions 0/32/64/96) so the array
     pipelines multiple matmuls concurrently; j-half x h-half go to
     partition quarters of one PSUM tile; evacuate halves on DVE overlapped
     with the output DMA.
Host sums the two partial cores per batch and applies swish.
"""

import sys

for _p in ("/opt/trn_rl_repo",):
    if _p not in sys.path:
        sys.path.append(_p)

import numpy as np

import concourse.bacc as bacc
import concourse.mybir as mybir
import concourse.tile as tile
from concourse.bass_utils import run_bass_kernel_spmd

B, T, D, HS = 4, 4096, 512, 64
TH = T // 2          # query rows per core
P = 128              # partitions
NB = TH // P         # 16 i-blocks per core
NJ = 512             # matmul moving free dim
VSCALE = 1024.0      # fp16 dynamic-range scale folded into V/l
DVE_J = 3520         # columns with DVE fused mask+rowsum (stt, 1x); rest:
                     # DVE tensor_tensor (2x) + ACT copy-accum row-sum

F16 = mybir.dt.float16
F32 = mybir.dt.float32
AF = mybir.ActivationFunctionType
ALU = mybir.AluOpType

_CACHE = {}


def _build():
    if "nc" in _CACHE:
        return _CACHE["nc"]

    nc = bacc.Bacc("TRN2", target_bir_lowering=False, debug=False)

    kt_d = nc.dram_tensor("kt", [HS, T], F16, kind="ExternalInput").ap()
    qt_d = nc.dram_tensor("qt", [HS, TH], F16, kind="ExternalInput").ap()
    vd_d = nc.dram_tensor("vd", [P, NB * HS], F16, kind="ExternalInput").ap()
    m_d = nc.dram_tensor("m", [TH, T], F16, kind="ExternalInput").ap()
    ot_d = nc.dram_tensor("ot", [P, TH], F16, kind="ExternalOutput").ap()

    with tile.TileContext(nc) as tc:
        with tc.tile_pool(name="persist", bufs=1) as pp:
            kt = pp.tile([HS, T], F16, tag="kt")
            qt = pp.tile([HS, TH], F16, tag="qt")
            v = pp.tile([P, NB * HS], F16, tag="v")
            vt = pp.tile([P, NB * HS], F16, tag="vt")
            u_all = pp.tile([P, NB * T], F16, tag="u_all")

            # kt on the Sync queue; qt/v in parallel on the Activation queue
            nc.sync.dma_start(kt[:, :NJ], kt_d[:, :NJ])
            nc.scalar.dma_start(qt[:], qt_d[:, :])
            nc.sync.dma_start(kt[:, NJ:TH], kt_d[:, NJ:TH])
            nc.sync.dma_start(kt[:, TH:], kt_d[:, TH:])
            nc.scalar.dma_start(v[:], vd_d[:, :])

            # ---- phase B: scores -> exp -> mask+rowsum -> vt, per block ----
            with (
                tc.tile_pool(name="psB", bufs=2, space="PSUM") as psB,
                tc.tile_pool(name="phB", bufs=2) as pB,
                tc.tile_pool(name="phM", bufs=3) as pM,
                tc.tile_pool(name="phBl", bufs=2) as pBl,
            ):
                def finish_block(k, l_d, l_a):
                    # combine row-sum parts, fold 1/l and VSCALE into vt
                    u = u_all[:, k * T:(k + 1) * T]
                    nc.scalar.activation(
                        dump[:], u[:, DVE_J:], AF.Copy, accum_out=l_a[:],
                    )
                    l_k = pBl.tile([P, 1], F32, tag="l_k", name="l_k")
                    nc.vector.tensor_tensor(out=l_k[:], in0=l_d[:],
                                            in1=l_a[:], op=ALU.add)
                    rl = pBl.tile([P, 1], F32, tag="rl", name="rl")
                    nc.vector.reciprocal(rl[:], l_k[:])
                    nc.vector.tensor_scalar(
                        out=vt[:, k * HS:(k + 1) * HS],
                        in0=v[:, k * HS:(k + 1) * HS],
                        scalar1=rl[:],
                        scalar2=VSCALE,
                        op0=ALU.mult,
                        op1=ALU.mult,
                    )

                dump = pBl.tile([P, T - DVE_J], F16, tag="dump")
                pend = None  # (k, l_d, l_a) awaiting ACT copy-accum sum
                for k in range(NB):
                    mk = pM.tile([P, T], F16, tag="mask")
                    nc.sync.dma_start(mk[:], m_d[k * P:(k + 1) * P, :])
                    er = pB.tile([P, T], F16, tag="er")
                    for jh in range(2):
                        sp = psB.tile([P, TH], F32, tag="sp")
                        for q2 in range(4):
                            j0 = jh * TH + q2 * NJ
                            nc.tensor.matmul(
                                sp[:, q2 * NJ:(q2 + 1) * NJ],
                                qt[:, k * P:(k + 1) * P],
                                kt[:, j0:j0 + NJ],
                                start=True,
                                stop=True,
                            )
                        nc.scalar.activation(
                            er[:, jh * TH:(jh + 1) * TH], sp[:],
                            AF.Exp, scale=0.125,
                        )
                    u = u_all[:, k * T:(k + 1) * T]
                    l_d = pBl.tile([P, 1], F32, tag="l_d", name="l_d")
                    l_a = pBl.tile([P, 1], F32, tag="l_a", name="l_a")
                    nc.vector.scalar_tensor_tensor(
                        out=u[:, :DVE_J], in0=er[:, :DVE_J], scalar=1.0,
                        in1=mk[:, :DVE_J], op0=ALU.mult, op1=ALU.mult,
                        accum_out=l_d[:],
                    )
                    nc.vector.tensor_tensor(
                        out=u[:, DVE_J:], in0=er[:, DVE_J:],
                        in1=mk[:, DVE_J:], op=ALU.mult,
                    )
                    if pend is not None:
                        finish_block(*pend)
                    pend = (k, l_d, l_a)
                finish_block(*pend)

            # ---- phase C: AV on four PE column quadrants (explicit tile
            # positions 0/32/64/96); partition quarter (hf*2+hh) holds
            # j-half hf, h-quarter hh ----
            with tc.tile_pool(name="psOT", bufs=1, space="PSUM") as psOT:
                ot_ps = psOT.tile([P, TH], F32, tag="ot")
                for k in range(NB):
                    for hf in range(2):
                        for hh in range(2):
                            qd = (hf * 2 + hh) * 32
                            for q2 in range(4):
                                nc.tensor.matmul(
                                    ot_ps[qd:qd + 32,
                                          q2 * NJ:(q2 + 1) * NJ],
                                    vt[:, k * HS + hh * 32:
                                       k * HS + hh * 32 + 32],
                                    u_all[:, k * T + hf * TH + q2 * NJ:
                                          k * T + hf * TH + (q2 + 1) * NJ],
                                    start=(k == 0),
                                    stop=(k == NB - 1),
                                    tile_position=(0, qd),
                                )
                with tc.tile_pool(name="phC", bufs=1) as pC:
                    ot_sb = pC.tile([P, TH], F16, tag="ot_sb")
                    for ch in range(4):
                        cs = slice(ch * NJ, (ch + 1) * NJ)
                        if ch % 2 == 0:
                            nc.vector.tensor_copy(ot_sb[:, cs], ot_ps[:, cs])
                            nc.sync.dma_start(ot_d[:, cs], ot_sb[:, cs])
                        else:
                            nc.scalar.copy(ot_sb[:, cs], ot_ps[:, cs])
                            nc.scalar.dma_start(ot_d[:, cs], ot_sb[:, cs])

    nc.compile()
    _CACHE["nc"] = nc
    return nc


def _prep_inputs(X, dag, Wk, Wq, Wv):
    X = np.asarray(X, dtype=np.float32)
    dag = np.asarray(dag)
    Wk = np.asarray(Wk, dtype=np.float32)
    Wq = np.asarray(Wq, dtype=np.float32)
    Wv = np.asarray(Wv, dtype=np.float32)
    m16 = (dag != 0).astype(np.float16)
    in_maps = []
    for b in range(B):
        K = (X[b] @ Wk).astype(np.float16)   # [T, HS]
        Q = (X[b] @ Wq).astype(np.float16)
        V = (X[b] @ Wv).astype(np.float16)
        for h in range(2):
            kt_full = K.T  # [HS, T]
            kt_rot = np.concatenate(
                [kt_full[:, h * TH:], kt_full[:, :h * TH]], axis=1
            )
            m_h = m16[h * TH:(h + 1) * TH]
            m_rot = np.concatenate([m_h[:, h * TH:], m_h[:, :h * TH]], axis=1)
            v_h = V[h * TH:(h + 1) * TH]  # [TH, HS]
            v_packed = np.ascontiguousarray(
                v_h.reshape(NB, P, HS).transpose(1, 0, 2).reshape(P, NB * HS)
            )
            in_maps.append(
                {
                    "kt": np.ascontiguousarray(kt_rot),
                    "qt": np.ascontiguousarray(Q.T[:, h * TH:(h + 1) * TH]),
                    "vd": v_packed,
                    "m": np.ascontiguousarray(m_rot),
                }
            )
    return in_maps


def kernel(X, dag, Wk, Wq, Wv, _trace=False):
    nc = _build()
    in_maps = _prep_inputs(X, dag, Wk, Wq, Wv)
    res = run_bass_kernel_spmd(nc, in_maps, list(range(8)), trace=_trace)
    out = np.empty((B, T, HS), dtype=np.float32)
    for b in range(B):
        acc = np.zeros((HS, T), dtype=np.float32)
        for h in range(2):
            ot = res.results[2 * b + h]["ot"].astype(np.float32)  # [128, TH]
            o_rot = np.empty((HS, T), dtype=np.float32)
            for hf in range(2):
                for hh in range(2):
                    q = (hf * 2 + hh) * 32
                    o_rot[hh * 32:(hh + 1) * 32, hf * TH:(hf + 1) * TH] = \
                        ot[q:q + 32]
            acc += np.roll(o_rot, h * TH, axis=1)
        o = acc.T / np.float32(VSCALE)
        out[b] = o / (1.0 + np.exp(-o))  # swish: o * sigmoid(o)
    if _trace:
        return out, res
    return out


# revision 37
# speedup vs baseline: 1.0051x; 1.0051x over previous
"""Trainium2 Bass kernel for nn_Head (sparse attention head).

Computation (per batch b):
    K = X @ Wk; Q = X @ Wq; V = X @ Wv                       # [T, HS]
    S = Q K^T / sqrt(HS)                                     # [T, T]
    A = softmax_row(where(dag==0, -inf, S))                  # row-wise over keys
    out[j, h] = sum_i A[i, j] V[i, h]   (transposed AV)      # [T, HS]
    return swish(out)

Sharding over 8 NeuronCores: core = (b, h) with b = batch (4), h = query-row
half (2).  The host computes the cheap O(T*D*HS) projections (4% of FLOPs)
and ships K^T/Q^T/V per core; the device does the O(T^2) work.  The key axis
is rotated per core by h*TH so DMA layouts are core-independent; the host
un-rotates the output.

Device phases per core:
  B: per 128-query block: QK matmuls (fp16, contraction 64) -> exp on ACT
     (scale 1/8) -> u = er*mask with the row-sum split three ways: DVE
     fused scalar_tensor_tensor for j < DVE_J, DVE tensor_tensor (2x mode)
     for the rest, whose sum comes from an ACT copy-accum pass (emitted one
     block late so it never stalls the exp stream) -> fold 1/l and VSCALE
     into vt.
  C: transposed-AV matmuls, PE-only, as four 32-wide output streams on
     distinct PE column quadrants (tile positions 0/32/64/96) so the array
     pipelines multiple matmuls concurrently; j-half x h-half go to
     partition quarters of one PSUM tile; evacuate halves on DVE overlapped
     with the output DMA.
Host sums the two partial cores per batch and applies swish.
"""

import sys

for _p in ("/opt/trn_rl_repo",):
    if _p not in sys.path:
        sys.path.append(_p)

import numpy as np

import concourse.bacc as bacc
import concourse.mybir as mybir
import concourse.tile as tile
from concourse.bass_utils import run_bass_kernel_spmd

B, T, D, HS = 4, 4096, 512, 64
TH = T // 2          # query rows per core
P = 128              # partitions
NB = TH // P         # 16 i-blocks per core
NJ = 512             # matmul moving free dim
VSCALE = 1024.0      # fp16 dynamic-range scale folded into V/l
DVE_J = 3456         # columns with DVE fused mask+rowsum (stt, 1x); rest:
                     # DVE tensor_tensor (2x) + ACT copy-accum row-sum

F16 = mybir.dt.float16
F32 = mybir.dt.float32
AF = mybir.ActivationFunctionType
ALU = mybir.AluOpType

_CACHE = {}


def _build():
    if "nc" in _CACHE:
        return _CACHE["nc"]

    nc = bacc.Bacc("TRN2", target_bir_lowering=False, debug=False)

    kt_d = nc.dram_tensor("kt", [HS, T], F16, kind="ExternalInput").ap()
    qt_d = nc.dram_tensor("qt", [HS, TH], F16, kind="ExternalInput").ap()
    vd_d = nc.dram_tensor("vd", [P, NB * HS], F16, kind="ExternalInput").ap()
    m_d = nc.dram_tensor("m", [TH, T], F16, kind="ExternalInput").ap()
    ot_d = nc.dram_tensor("ot", [P, TH], F16, kind="ExternalOutput").ap()

    with tile.TileContext(nc) as tc:
        with tc.tile_pool(name="persist", bufs=1) as pp:
            kt = pp.tile([HS, T], F16, tag="kt")
            qt = pp.tile([HS, TH], F16, tag="qt")
            v = pp.tile([P, NB * HS], F16, tag="v")
            vt = pp.tile([P, NB * HS], F16, tag="vt")
            u_all = pp.tile([P, NB * T], F16, tag="u_all")

            # kt on the Sync queue; qt/v in parallel on the Activation queue
            nc.sync.dma_start(kt[:, :NJ], kt_d[:, :NJ])
            nc.scalar.dma_start(qt[:], qt_d[:, :])
            nc.sync.dma_start(kt[:, NJ:TH], kt_d[:, NJ:TH])
            nc.sync.dma_start(kt[:, TH:], kt_d[:, TH:])
            nc.scalar.dma_start(v[:], vd_d[:, :])

            # ---- phase B: scores -> exp -> mask+rowsum -> vt, per block ----
            with (
                tc.tile_pool(name="psB", bufs=2, space="PSUM") as psB,
                tc.tile_pool(name="phB", bufs=2) as pB,
                tc.tile_pool(name="phM", bufs=3) as pM,
                tc.tile_pool(name="phBl", bufs=2) as pBl,
            ):
                def finish_block(k, l_d, l_a):
                    # combine row-sum parts, fold 1/l and VSCALE into vt
                    u = u_all[:, k * T:(k + 1) * T]
                    nc.scalar.activation(
                        dump[:], u[:, DVE_J:], AF.Copy, accum_out=l_a[:],
                    )
                    l_k = pBl.tile([P, 1], F32, tag="l_k", name="l_k")
                    nc.vector.tensor_tensor(out=l_k[:], in0=l_d[:],
                                            in1=l_a[:], op=ALU.add)
                    rl = pBl.tile([P, 1], F32, tag="rl", name="rl")
                    nc.vector.reciprocal(rl[:], l_k[:])
                    nc.vector.tensor_scalar(
                        out=vt[:, k * HS:(k + 1) * HS],
                        in0=v[:, k * HS:(k + 1) * HS],
                        scalar1=rl[:],
                        scalar2=VSCALE,
                        op0=ALU.mult,
                        op1=ALU.mult,
                    )

                dump = pBl.tile([P, T - DVE_J], F16, tag="dump")
                pend = None  # (k, l_d, l_a) awaiting ACT copy-accum sum
                for k in range(NB):
                    mk = pM.tile([P, T], F16, tag="mask")
                    nc.sync.dma_start(mk[:], m_d[k * P:(k + 1) * P, :])
                    er = pB.tile([P, T], F16, tag="er")
                    for jh in range(2):
                        sp = psB.tile([P, TH], F32, tag="sp")
                        for q2 in range(4):
                            j0 = jh * TH + q2 * NJ
                            nc.tensor.matmul(
                                sp[:, q2 * NJ:(q2 + 1) * NJ],
                                qt[:, k * P:(k + 1) * P],
                                kt[:, j0:j0 + NJ],
                                start=True,
                                stop=True,
                            )
                        nc.scalar.activation(
                            er[:, jh * TH:(jh + 1) * TH], sp[:],
                            AF.Exp, scale=0.125,
                        )
                    u = u_all[:, k * T:(k + 1) * T]
                    l_d = pBl.tile([P, 1], F32, tag="l_d", name="l_d")
                    l_a = pBl.tile([P, 1], F32, tag="l_a", name="l_a")
                    nc.vector.scalar_tensor_tensor(
                        out=u[:, :DVE_J], in0=er[:, :DVE_J], scalar=1.0,
                        in1=mk[:, :DVE_J], op0=ALU.mult, op1=ALU.mult,
                        accum_out=l_d[:],
                    )
                    nc.vector.tensor_tensor(
                        out=u[:, DVE_J:], in0=er[:, DVE_J:],
                        in1=mk[:, DVE_J:], op=ALU.mult,
                    )
                    if pend is not None:
                        finish_block(*pend)
                    pend = (k, l_d, l_a)
                finish_block(*pend)

            # ---- phase C: AV on four PE column quadrants (explicit tile
            # positions 0/32/64/96); partition quarter (hf*2+hh) holds
            # j-half hf, h-quarter hh ----
            with tc.tile_pool(name="psOT", bufs=1, space="PSUM") as psOT:
                ot_ps = psOT.tile([P, TH], F32, tag="ot")
                for k in range(NB):
                    for hf in range(2):
                        for hh in range(2):
                            qd = (hf * 2 + hh) * 32
                            for q2 in range(4):
                                nc.tensor.matmul(
                                    ot_ps[qd:qd + 32,
                                          q2 * NJ:(q2 + 1) * NJ],
                                    vt[:, k * HS + hh * 32:
                                       k * HS + hh * 32 + 32],
                                    u_all[:, k * T + hf * TH + q2 * NJ:
                                          k * T + hf * TH + (q2 + 1) * NJ],
                                    start=(k == 0),
                                    stop=(k == NB - 1),
                                    tile_position=(0, qd),
                                )
                with tc.tile_pool(name="phC", bufs=1) as pC:
                    ot_sb = pC.tile([P, TH], F16, tag="ot_sb")
                    for ch in range(4):
                        cs = slice(ch * NJ, (ch + 1) * NJ)
                        if ch % 2 == 0:
                            nc.vector.tensor_copy(ot_sb[:, cs], ot_ps[:, cs])
                            nc.sync.dma_start(ot_d[:, cs], ot_sb[:, cs])
                        else:
                            nc.scalar.copy(ot_sb[:, cs], ot_ps[:, cs])
                            nc.scalar.dma_start(ot_d[:, cs], ot_sb[:, cs])

    nc.compile()
    _CACHE["nc"] = nc
    return nc


def _prep_inputs(X, dag, Wk, Wq, Wv):
    X = np.asarray(X, dtype=np.float32)
    dag = np.asarray(dag)
    Wk = np.asarray(Wk, dtype=np.float32)
    Wq = np.asarray(Wq, dtype=np.float32)
    Wv = np.asarray(Wv, dtype=np.float32)
    m16 = (dag != 0).astype(np.float16)
    in_maps = []
    for b in range(B):
        K = (X[b] @ Wk).astype(np.float16)   # [T, HS]
        Q = (X[b] @ Wq).astype(np.float16)
        V = (X[b] @ Wv).astype(np.float16)
        for h in range(2):
            kt_full = K.T  # [HS, T]
            kt_rot = np.concatenate(
                [kt_full[:, h * TH:], kt_full[:, :h * TH]], axis=1
            )
            m_h = m16[h * TH:(h + 1) * TH]
            m_rot = np.concatenate([m_h[:, h * TH:], m_h[:, :h * TH]], axis=1)
            v_h = V[h * TH:(h + 1) * TH]  # [TH, HS]
            v_packed = np.ascontiguousarray(
                v_h.reshape(NB, P, HS).transpose(1, 0, 2).reshape(P, NB * HS)
            )
            in_maps.append(
                {
                    "kt": np.ascontiguousarray(kt_rot),
                    "qt": np.ascontiguousarray(Q.T[:, h * TH:(h + 1) * TH]),
                    "vd": v_packed,
                    "m": np.ascontiguousarray(m_rot),
                }
            )
    return in_maps


def kernel(X, dag, Wk, Wq, Wv, _trace=False):
    nc = _build()
    in_maps = _prep_inputs(X, dag, Wk, Wq, Wv)
    res = run_bass_kernel_spmd(nc, in_maps, list(range(8)), trace=_trace)
    out = np.empty((B, T, HS), dtype=np.float32)
    for b in range(B):
        acc = np.zeros((HS, T), dtype=np.float32)
        for h in range(2):
            ot = res.results[2 * b + h]["ot"].astype(np.float32)  # [128, TH]
            o_rot = np.empty((HS, T), dtype=np.float32)
            for hf in range(2):
                for hh in range(2):
                    q = (hf * 2 + hh) * 32
                    o_rot[hh * 32:(hh + 1) * 32, hf * TH:(hf + 1) * TH] = \
                        ot[q:q + 32]
            acc += np.roll(o_rot, h * TH, axis=1)
        o = acc.T / np.float32(VSCALE)
        out[b] = o / (1.0 + np.exp(-o))  # swish: o * sigmoid(o)
    if _trace:
        return out, res
    return out
